# revision 33
# baseline (speedup 1.0000x reference)
"""Trainium2 Bass kernel for nn_Alignment: bidirectional masked softmax attention.

reference:
  scores = einsum('bld,bmd->blm', a, b) * temp              [B, La, Lb]
  mask   = outer(mask_a, mask_b) > 0;  scores = where(mask, scores, -1e4)
  attention_a = softmax(scores, axis=1); attention_b = softmax(scores, axis=2)
  feature_a = attention_b @ b;  feature_b = attention_a @ a
  returns (feature_a, feature_b)

v3: data-parallel over batch, 4 examples/core.  Same math as the v2 baseline
(T[m,l] = temp*ma_l*(b@a^T) layout; E = exp(temp*T + negB) with rowsums from
the activation accumulator; masked-row/col values excised arithmetically and
patched with closed-form uniform-softmax terms), rescheduled around the
CoreSim v1 cost model where PE (~49us) and the serial exp chain (~40us) are
the co-critical resources:
  - a/rowsum written with a single free-dim-broadcast multiply (1 DVE op
    instead of 32 TensorScalarPtr) so stage-2 unblocks ~5us sooner.
  - examples 0/1 get their transposed operands from PE transposes (identity
    matmuls; PSUM read-back on DVE/ACT - gpsimd cannot touch PSUM on HW)
    so the first scores matmul issues ~4.5us in; examples 2/3 use the
    DRAM-staged xbar DMA transpose, prefetched ~2 slots ahead.
  - full-tile DMAs only (the v1 model floors every DMA at 500ns and charges
    transfer time on the issuing engine queue), spread across SP/ACT/Pool.
  - steady slots interleave stage1(e) at exp cadence with stage2(e-1) skewed
    3 steps (its mask-independent [b|1] half pre-filled 1 step earlier), so
    the rowsum-fix latency and slot boundaries stay off the PE queue's path.
  - temperature applied via the exp scale operand, not a separate multiply.
CoreSim: 62116 ns/core (baseline 72847); silicon rel err ~3.2e-3.
"""

import numpy as np

import concourse.bass as bass
import concourse.bacc as bacc
import concourse.tile as tile
from concourse import mybir
from concourse.bass_utils import run_bass_kernel_spmd

B, L, H = 32, 1024, 128
NCORES = 8
EPB = B // NCORES  # examples per core
NT = L // 128      # 128-row tiles per sequence

f32 = mybir.dt.float32
bf16 = mybir.dt.bfloat16
i32 = mybir.dt.int32
AF = mybir.ActivationFunctionType
ALU = mybir.AluOpType
AX = mybir.AxisListType

NEG = -120.0  # exp(x + NEG) flushes to 0 in bf16 for any reachable score x
W = 2 * H + 1  # stage-2 rhs width [b | 1 | a/rowsum]
N_WARM = 2


def build_nc() -> bass.Bass:
    nc = bacc.Bacc(None, target_bir_lowering=False)
    a_ext = nc.declare_dram_parameter("a", [EPB, L, H], f32, isOutput=False)
    b_ext = nc.declare_dram_parameter("b", [EPB, L, H], f32, isOutput=False)
    ma_ext = nc.declare_dram_parameter("mask_a", [EPB, L, 1], i32, isOutput=False)
    mb_ext = nc.declare_dram_parameter("mask_b", [EPB, L, 1], i32, isOutput=False)
    t_ext = nc.declare_dram_parameter("temperature", [1, 1], f32, isOutput=False)
    out_ext = nc.declare_dram_parameter("out", [2, EPB, L, H], f32, isOutput=True)

    with tile.TileContext(nc) as tc:
        with (
            tc.tile_pool(name="const", bufs=1) as const,
            tc.tile_pool(name="sm", bufs=3) as sm,
            tc.tile_pool(name="big", bufs=3) as big,
            tc.tile_pool(name="ebuf", bufs=4) as ebuf,
            tc.tile_pool(name="post", bufs=3) as post,
            tc.tile_pool(name="dr", bufs=2, space="DRAM") as dr,
            tc.tile_pool(name="ps", bufs=2, space="PSUM") as ps,
            tc.tile_pool(name="ps2", bufs=4, space="PSUM") as ps2,
        ):
            # ---------------- constants ----------------
            temp_col = const.tile([128, 1], f32)
            ones1 = const.tile([1, 128], bf16)
            nc.vector.memset(ones1[:], 1.0)
            ones1f = const.tile([1, 128], f32)
            nc.vector.memset(ones1f[:], 1.0)
            onesc = const.tile([128, 1], bf16)
            nc.vector.memset(onesc[:], 1.0)
            ones128 = const.tile([128, 128], bf16)
            nc.vector.memset(ones128[:], 1.0)
            ident16 = const.tile([128, 128], bf16)
            nc.gpsimd.affine_select(
                out=ident16[:], in_=ones128[:], pattern=[[-1, 128]],
                compare_op=ALU.is_equal, fill=0.0, base=0, channel_multiplier=1,
            )
            ident32 = const.tile([128, 128], f32)
            nc.vector.tensor_copy(out=ident32[:], in_=ident16[:])

            # warmup matmuls: keep PE busy from t~0 so the pstate ramp
            # finishes during input-DMA latency.
            for i in range(N_WARM):
                warm_ps = ps2.tile([128, 128], f32, tag="o2", name="warm_ps")
                nc.tensor.matmul(
                    warm_ps[:], lhsT=ones1[:], rhs=ones1[:], start=True, stop=True
                )

            # ---------------- per-example state ----------------
            ma_f = {}
            mb_f = {}
            oma = {}
            omb = {}
            cw = {}
            nmA_col = {}
            negB = {}
            a_f = {}
            b_f = {}
            st = {}
            ab_d = {}
            aT = {}
            bT = {}
            comb = {}
            E_all = {}
            rs_raw = {}
            recip = {}
            corr_bc = {}
            fa_all = {}
            fb_all = {}

            def emit_mask_loads(e, eng):
                ma_i = sm.tile([128, NT], i32, tag="ma_i", name="ma_i")
                eng.dma_start(
                    out=ma_i[:], in_=ma_ext[e, :, 0].rearrange("(r p) -> p r", p=128)
                )
                mb_i = sm.tile([128, NT], i32, tag="mb_i", name="mb_i")
                eng.dma_start(
                    out=mb_i[:], in_=mb_ext[e, :, 0].rearrange("(r p) -> p r", p=128)
                )
                return ma_i, mb_i

            def emit_mask_convert(e, ma_i, mb_i):
                ma_f[e] = sm.tile([128, NT], f32, tag="ma_f", name="ma_f")
                nc.gpsimd.tensor_copy(out=ma_f[e][:], in_=ma_i[:])
                mb_f[e] = sm.tile([128, NT], f32, tag="mb_f", name="mb_f")
                nc.gpsimd.tensor_copy(out=mb_f[e][:], in_=mb_i[:])

            def emit_smalls(e):
                # negB = (mb - 1) * 120  -> 0 (valid) / -120 (masked)
                negB[e] = sm.tile([128, NT], f32, tag="negB", name="negB")
                nc.vector.tensor_scalar(
                    out=negB[e][:], in0=mb_f[e][:], scalar1=1.0, scalar2=-NEG,
                    op0=ALU.subtract, op1=ALU.mult,
                )
                omb[e] = sm.tile([128, NT], f32, tag="omb", name="omb")
                nc.vector.tensor_scalar(
                    out=omb[e][:], in0=mb_f[e][:], scalar1=-1.0, scalar2=1.0,
                    op0=ALU.mult, op1=ALU.add,
                )
                oma[e] = sm.tile([128, NT], f32, tag="oma", name="oma")
                nc.vector.tensor_scalar(
                    out=oma[e][:], in0=ma_f[e][:], scalar1=-1.0, scalar2=1.0,
                    op0=ALU.mult, op1=ALU.add,
                )
                cw[e] = sm.tile([128, NT, 2], bf16, tag="cw", name="cw")
                nc.vector.memset(cw[e][:, :, 0], 1.0 / L)
                nc.vector.tensor_scalar(
                    out=cw[e][:, :, 1], in0=omb[e][:], scalar1=1.0 / L,
                    scalar2=None, op0=ALU.mult,
                )

            ppc_bf_d = {}

            def emit_nmA_pre(e):
                ppc = sm.tile([128, 1], f32, tag="ppc", name="ppc")
                nc.vector.reduce_sum(out=ppc[:], in_=ma_f[e][:], axis=AX.X)
                ppc_bf_d[e] = sm.tile([128, 1], bf16, tag="ppc_bf", name="ppc_bf")
                nc.vector.tensor_copy(out=ppc_bf_d[e][:], in_=ppc[:])

            def emit_nmA(e):
                # negnmA = sum(mask_a) - 1024 broadcast to a column
                cnt_ps = ps2.tile([1, 1], f32, tag="o2", name="cnt_ps")
                nc.tensor.matmul(
                    cnt_ps[:], lhsT=ppc_bf_d[e][:], rhs=onesc[:], start=True,
                    stop=True,
                )
                nmA = sm.tile([1, 1], f32, tag="nmA", name="nmA")
                nc.vector.tensor_scalar(
                    out=nmA[:], in0=cnt_ps[:], scalar1=-float(L), scalar2=None,
                    op0=ALU.add,
                )
                bc2_ps = ps2.tile([128, 1], f32, tag="o2", name="bc2_ps")
                nc.tensor.matmul(
                    bc2_ps[:], lhsT=ones1f[:], rhs=nmA[:], start=True, stop=True
                )
                nmA_col[e] = sm.tile([128, 1], f32, tag="nmA_col", name="nmA_col")
                nc.vector.tensor_copy(out=nmA_col[e][:], in_=bc2_ps[:])

            def alloc_af(e):
                a_f[e] = big.tile([128, NT, H], f32, tag="a_f", name="a_f")
                b_f[e] = big.tile([128, NT, H], f32, tag="b_f", name="b_f")

            def emit_load_q(e, q, eng_a, eng_b):
                rows = slice(q * 256, (q + 1) * 256)
                rs_ = slice(q * 2, (q + 1) * 2)
                eng_b.dma_start(
                    out=b_f[e][:, rs_, :],
                    in_=b_ext[e, rows].rearrange("(r p) d -> p r d", p=128),
                )
                eng_a.dma_start(
                    out=a_f[e][:, rs_, :],
                    in_=a_ext[e, rows].rearrange("(r p) d -> p r d", p=128),
                )

            def emit_load_full(e, eng_a, eng_b):
                eng_a.dma_start(
                    out=a_f[e][:], in_=a_ext[e].rearrange("(r p) d -> p r d", p=128)
                )
                eng_b.dma_start(
                    out=b_f[e][:], in_=b_ext[e].rearrange("(r p) d -> p r d", p=128)
                )

            def alloc_xops(e):
                aT[e] = big.tile([128, L], bf16, tag="aT", name="aT")
                bT[e] = big.tile([128, L], bf16, tag="bT", name="bT")

            def emit_sta(e, r, eng=None):
                # st a-part: a * mask_a, cast to bf16 (needed pre-transpose)
                if e not in st:
                    st[e] = big.tile([128, NT, 2 * H], bf16, tag="st", name="st")
                (eng or nc.gpsimd).tensor_tensor(
                    out=st[e][:, r, 0:H], in0=a_f[e][:, r, :],
                    in1=ma_f[e][:, r : r + 1].to_broadcast([128, H]),
                    op=ALU.mult,
                )

            def emit_stb(e, h):
                hs = slice(h * (NT // 2), (h + 1) * (NT // 2))
                nc.gpsimd.tensor_copy(
                    out=st[e][:, hs, H : 2 * H], in_=b_f[e][:, hs, :]
                )

            def emit_xp_a(e, j, cp=None):
                # PE transpose of st a-part tile j -> aT[:, 128j:128j+128]
                # (PSUM readers must be DVE/ACT on real HW; gpsimd is illegal)
                tp16 = ps2.tile([128, 128], bf16, tag="o2", name="tp16")
                nc.tensor.transpose(tp16[:], in_=st[e][:, j, 0:H], identity=ident16[:])
                if cp is nc.scalar:
                    nc.scalar.activation(
                        out=aT[e][:, j * 128 : (j + 1) * 128], in_=tp16[:],
                        func=AF.Copy,
                    )
                else:
                    (cp or nc.vector).tensor_copy(
                        out=aT[e][:, j * 128 : (j + 1) * 128], in_=tp16[:]
                    )

            def emit_xp_b(e, j, cp=None):
                # PE transpose of raw b_f tile j (f32) -> bT bf16
                tp32 = ps2.tile([128, 128], f32, tag="o2", name="tp32")
                nc.tensor.transpose(tp32[:], in_=b_f[e][:, j, :], identity=ident32[:])
                if cp is nc.scalar:
                    nc.scalar.activation(
                        out=bT[e][:, j * 128 : (j + 1) * 128], in_=tp32[:],
                        func=AF.Copy,
                    )
                else:
                    (cp or nc.vector).tensor_copy(
                        out=bT[e][:, j * 128 : (j + 1) * 128], in_=tp32[:]
                    )

            def emit_abd(e, eng):
                ab_d[e] = dr.tile([L, 2 * H], bf16, tag="ab_d", name="ab_d")
                eng.dma_start(
                    out=ab_d[e][:].rearrange("(r p) d -> p r d", p=128),
                    in_=st[e][:],
                )

            def emit_xbar(e, h, eng):
                rows = slice(h * (L // 2), (h + 1) * (L // 2))
                eng.dma_start_transpose(out=aT[e][:, rows], in_=ab_d[e][rows, 0:H])
                eng.dma_start_transpose(
                    out=bT[e][:, rows], in_=ab_d[e][rows, H : 2 * H]
                )

            def emit_comb_static(e):
                comb[e] = big.tile([128, NT, W], bf16, tag="comb", name="comb")
                nc.gpsimd.tensor_copy(out=comb[e][:, :, 0:H], in_=b_f[e][:])
                nc.gpsimd.memset(comb[e][:, :, H : H + 1], 1.0)

            def emit_stage1(e, r, c):
                if r == 0 and c == 0:
                    E_all[e] = ebuf.tile([128, NT, L], bf16, tag="E", name="E_all")
                if c == 0:
                    tps = ps.tile([128, L], f32, tag="T", name="t_ps")
                    emit_stage1.tps[(e, r)] = tps
                tps = emit_stage1.tps[(e, r)]
                nc.tensor.matmul(
                    tps[:, c * 512 : (c + 1) * 512],
                    lhsT=bT[e][:, r * 128 : (r + 1) * 128],
                    rhs=aT[e][:, c * 512 : (c + 1) * 512],
                    start=True, stop=True,
                )

            emit_stage1.tps = {}

            rs_h = {}

            def emit_exp(e, r, half=None):
                if r == 0 and half in (None, 0):
                    rs_raw[e] = sm.tile([128, NT], f32, tag="rs_raw", name="rs_raw")
                if half is None:
                    tps = emit_stage1.tps.pop((e, r))
                    nc.scalar.activation(
                        out=E_all[e][:, r, :], in_=tps[:],
                        func=AF.Exp, bias=negB[e][:, r : r + 1], scale=temp_col[:],
                        accum_out=rs_raw[e][:, r : r + 1],
                    )
                    return
                tps = emit_stage1.tps[(e, r)]
                if half == 0:
                    rs_h[e] = sm.tile([128, 1], f32, tag="rs_h", name="rs_h")
                    nc.scalar.activation(
                        out=E_all[e][:, r, 0:512], in_=tps[:, 0:512],
                        func=AF.Exp, bias=negB[e][:, r : r + 1], scale=temp_col[:],
                        accum_out=rs_h[e][:],
                    )
                else:
                    emit_stage1.tps.pop((e, r))
                    nc.scalar.activation(
                        out=E_all[e][:, r, 512:1024], in_=tps[:, 512:1024],
                        func=AF.Exp, bias=negB[e][:, r : r + 1], scale=temp_col[:],
                        accum_out=rs_raw[e][:, r : r + 1],
                    )
                    # rs_raw[:, r] += rs_h later in fix via stored handle


            def emit_fix(e):
                # rs = rs_raw + mb*negnmA + (1-mb); recip = 1/rs
                if e in rs_h:
                    nc.vector.tensor_tensor(
                        out=rs_raw[e][:, 0:1], in0=rs_raw[e][:, 0:1],
                        in1=rs_h[e][:], op=ALU.add,
                    )
                rs_u = sm.tile([128, NT], f32, tag="rs_u", name="rs_u")
                nc.vector.scalar_tensor_tensor(
                    out=rs_u[:], in0=mb_f[e][:], scalar=nmA_col[e][:],
                    in1=rs_raw[e][:], op0=ALU.mult, op1=ALU.add,
                )
                nc.vector.tensor_tensor(
                    out=rs_u[:], in0=rs_u[:], in1=omb[e][:], op=ALU.add
                )
                recip[e] = sm.tile([128, NT], f32, tag="recip", name="recip")
                nc.vector.reciprocal(out=recip[e][:], in_=rs_u[:])
                # a' = a / rowsum -> comb[:, :, H+1:] in ONE broadcast multiply
                nc.vector.tensor_tensor(
                    out=comb[e][:, :, H + 1 : W], in0=a_f[e][:],
                    in1=recip[e][:].rearrange("p (r u) -> p r u", u=1)
                    .to_broadcast([128, NT, H]),
                    op=ALU.mult,
                )

            def emit_corr(e):
                # corrA = mean(b); corrB = sum over masked m of a / L
                # (rs-trick makes comb_a rows equal raw a at masked m)
                corrAB = sm.tile([1, 2 * H], bf16, tag="corrAB", name="corrAB")
                # corrA = mean(b) via DVE reduce over bT's free (m) axis,
                # transposed to a row with one K=.. wait -- tiny PE transpose
                cA_f = sm.tile([128, 1], f32, tag="cA_f", name="cA_f")
                nc.vector.reduce_sum(out=cA_f[:], in_=bT[e][:], axis=AX.X)
                cA_bf = sm.tile([128, 1], bf16, tag="cA_bf", name="cA_bf")
                nc.vector.tensor_scalar(
                    out=cA_bf[:], in0=cA_f[:], scalar1=1.0 / L, scalar2=None,
                    op0=ALU.mult,
                )
                cA_tp = ps.tile([1, 128], bf16, tag="T", name="cA_tp")
                nc.tensor.matmul(
                    cA_tp[:], lhsT=cA_bf[:], rhs=ident16[:], start=True,
                    stop=True, is_transpose=True,
                )
                nc.vector.tensor_copy(out=corrAB[:, 0:H], in_=cA_tp[:])
                corrB_ps = ps.tile([1, H], f32, tag="T", name="corrB_ps")
                for r in range(NT):
                    nc.tensor.matmul(
                        corrB_ps[:], lhsT=cw[e][:, r, 1:2],
                        rhs=comb[e][:, r, H + 1 : W],
                        start=(r == 0), stop=(r == NT - 1),
                    )
                nc.vector.tensor_copy(out=corrAB[:, H : 2 * H], in_=corrB_ps[:])
                bc_ps = ps.tile([128, 2 * H], f32, tag="T", name="bc_ps")
                nc.tensor.matmul(
                    bc_ps[:], lhsT=ones1[:], rhs=corrAB[:], start=True, stop=True
                )
                corr_bc[e] = big.tile([128, 2 * H], f32, tag="corr_bc", name="corr_bc")
                nc.vector.tensor_copy(out=corr_bc[e][:], in_=bc_ps[:])

            def emit_stage2(e, lt, part=None):
                # part: None=both, 1=[b|1] cols (no a' dep), 2=a' cols
                if lt == 0 and part in (None, 1, "1a"):
                    fa_all[e] = big.tile([128, NT, H], f32, tag="fa", name="fa_all")
                    fb_all[e] = big.tile([128, NT, H], f32, tag="fb", name="fb_all")
                if part in (None, 1, "1a"):
                    o_ps = ps2.tile([128, W], f32, tag="o2", name="o_ps")
                    emit_stage2.ops[(e, lt)] = o_ps
                o_ps = emit_stage2.ops[(e, lt)]
                if part is None:
                    cols, rr = [slice(0, W)], range(NT)
                elif part == 1:
                    cols, rr = [slice(0, H + 1)], range(NT)
                elif part == "1a":
                    cols, rr = [slice(0, H + 1)], range(NT - 1)
                elif part == "1b":
                    cols, rr = [slice(0, H + 1)], range(NT - 1, NT)
                else:
                    cols, rr = [slice(H + 1, W)], range(NT)
                for cs in cols:
                    for r in rr:
                        nc.tensor.matmul(
                            o_ps[:, cs],
                            lhsT=E_all[e][:, r, lt * 128 : (lt + 1) * 128],
                            rhs=comb[e][:, r, cs],
                            start=(r == 0), stop=(r == NT - 1),
                        )

            emit_stage2.ops = {}

            def emit_epi(e, lt):
                o_ps = emit_stage2.ops.pop((e, lt))
                csum_r = post.tile([128, 1], f32, tag="csum_r", name="csum_r")
                nc.vector.reciprocal(out=csum_r[:], in_=o_ps[:, H : H + 1])
                # fa = (U_b/colsum)*mask_a + (1-mask_a)*corrA
                fa = fa_all[e][:, lt, :]
                nc.vector.tensor_scalar(
                    out=fa, in0=o_ps[:, 0:H], scalar1=csum_r[:],
                    scalar2=ma_f[e][:, lt : lt + 1], op0=ALU.mult, op1=ALU.mult,
                )
                fat = post.tile([128, H], f32, tag="fat", name="fat")
                nc.gpsimd.tensor_tensor(
                    out=fat[:], in0=corr_bc[e][:, 0:H],
                    in1=oma[e][:, lt : lt + 1].to_broadcast([128, H]),
                    op=ALU.mult,
                )
                nc.gpsimd.tensor_tensor(out=fa, in0=fa, in1=fat[:], op=ALU.add)
                # fb = U_a*mask_a + corrB
                nc.vector.scalar_tensor_tensor(
                    out=fb_all[e][:, lt, :], in0=o_ps[:, H + 1 : W],
                    scalar=ma_f[e][:, lt : lt + 1],
                    in1=corr_bc[e][:, H : 2 * H],
                    op0=ALU.mult, op1=ALU.add,
                )

            def emit_out(e, part=None):
                # part=None: full example; part=(h, nq): chunk h of nq
                if part is None:
                    nc.sync.dma_start(
                        out=out_ext[0, e].rearrange("(r p) d -> p r d", p=128),
                        in_=fa_all[e][:],
                    )
                    nc.sync.dma_start(
                        out=out_ext[1, e].rearrange("(r p) d -> p r d", p=128),
                        in_=fb_all[e][:],
                    )
                else:
                    h, nq = part
                    q = L // nq
                    qt = NT // nq
                    rows = slice(h * q, (h + 1) * q)
                    rt = slice(h * qt, (h + 1) * qt)
                    nc.gpsimd.dma_start(
                        out=out_ext[0, e, rows].rearrange("(r p) d -> p r d", p=128),
                        in_=fa_all[e][:, rt, :],
                    )
                    nc.sync.dma_start(
                        out=out_ext[1, e, rows].rearrange("(r p) d -> p r d", p=128),
                        in_=fb_all[e][:, rt, :],
                    )
                    return

            # ================= PROLOGUE (e0 + e1 via PE transposes) ========
            ma0_i, mb0_i = emit_mask_loads(0, nc.scalar)  # ACT queue
            nc.scalar.dma_start(out=temp_col[:], in_=t_ext[:].partition_broadcast(128))
            alloc_af(0)
            # b q0/q1 + all a quarters on SP; b q2/q3 on ACT (idle pre-exp)
            emit_load_q(0, 0, nc.sync, nc.sync)
            emit_load_q(0, 1, nc.sync, nc.scalar)
            emit_mask_convert(0, ma0_i, mb0_i)
            emit_smalls(0)
            emit_nmA_pre(0)
            emit_load_q(0, 2, nc.sync, nc.sync)
            emit_load_q(0, 3, nc.sync, nc.sync)
            alloc_xops(0)
            # quarters 0,1: startup-critical copies ride the pre-exp ACT window
            for q in (0, 1):
                for j in (2 * q, 2 * q + 1):
                    emit_xp_b(0, j, cp=(nc.scalar if j == 0 else None))
                    emit_sta(0, j)
                    emit_xp_a(0, j, cp=nc.scalar)
            # e1 input loads early (SP queue after e0 quarters)
            ma1_i, mb1_i = emit_mask_loads(1, nc.sync)
            alloc_af(1)
            emit_load_full(1, nc.sync, nc.sync)
            # first two scores matmuls (need aT halves 0..3 -> only c=0)
            emit_stage1(0, 0, 0)
            emit_stage1(0, 1, 0)
            for q in (2, 3):
                for j in (2 * q, 2 * q + 1):
                    emit_xp_b(0, j)
                    emit_sta(0, j)
                    emit_xp_a(0, j)
            emit_exp(0, 0, half=0)
            emit_stage1(0, 0, 1)
            emit_exp(0, 0, half=1)
            emit_stage1(0, 1, 1)
            emit_exp(0, 1)
            emit_mask_convert(1, ma1_i, mb1_i)
            emit_smalls(1)
            emit_nmA_pre(1)
            # ============ slot 0: stage1(0) + e1 PE transposes + e2 prep ===
            alloc_xops(1)
            for r in range(2, NT):
                emit_stage1(0, r, 0)
                emit_stage1(0, r, 1)
                emit_exp(0, r)
                if r == 2:
                    ma2_i, mb2_i = emit_mask_loads(2, nc.sync)
                    alloc_af(2)
                    emit_load_full(2, nc.sync, nc.sync)
                if r == 3:
                    for j in range(0, 4):
                        emit_xp_b(1, j, cp=nc.vector)
                if r == 4:
                    for j in range(4, NT):
                        emit_xp_b(1, j, cp=nc.vector)
                    for j in range(0, 4):
                        emit_sta(1, j)
                if r == 5:
                    emit_nmA(0)
                    for j in range(0, 4):
                        emit_xp_a(1, j, cp=nc.vector)
                    for j in range(4, NT):
                        emit_sta(1, j)
                if r == 6:
                    for j in range(4, NT):
                        emit_xp_a(1, j, cp=nc.vector)
                    emit_comb_static(0)
                if r == 7:
                    emit_mask_convert(2, ma2_i, mb2_i)
                    emit_smalls(2)
                    emit_nmA_pre(2)
            emit_fix(0)
            # e2 staging (DMA path)
            for j in range(NT):
                emit_sta(2, j)
            emit_stb(2, 0)
            emit_stb(2, 1)
            emit_comb_static(1)

            # ============ slots 1..3 ======================================
            def slot(e):
                """stage1(e)+exp(e); stage2(e-1) skewed by 2 steps; epi/out of
                e-2 at the head; prefetch pieces for e+2."""
                for r in range(NT):
                    emit_stage1(e, r, 0)
                    emit_stage1(e, r, 1)
                    emit_exp(e, r)
                    if r == 0 and e >= 2:
                        emit_epi(e - 2, NT - 2)
                    if r == 1 and e >= 2:
                        emit_epi(e - 2, NT - 1)
                    if r == 1:
                        if e == 1:
                            emit_nmA(1)
                        emit_stage2(e - 1, 0, part="1a")
                        emit_stage2(e - 1, 1, part="1a")
                    if r == 2 and e >= 2:
                        emit_out(e - 2)
                    if r == 2:
                        emit_stage2(e - 1, 0, part="1b")
                        emit_stage2(e - 1, 1, part="1b")
                    if r == 3:
                        emit_corr(e - 1)
                        emit_stage2(e - 1, 0, part=2)
                    if r == 4:
                        emit_stage2(e - 1, 1, part=2)
                    if r >= 4:
                        emit_epi(e - 1, r - 4)
                    if r >= 5:
                        emit_stage2(e - 1, r - 3)
                    # prefetch pieces
                    if e == 1:
                        if r == 0:
                            emit_abd(2, nc.gpsimd)
                            ma3_i, mb3_i = emit_mask_loads(3, nc.sync)
                            alloc_af(3)
                            emit_load_full(3, nc.sync, nc.sync)
                        if r == 2:
                            alloc_xops(2)
                            emit_xbar(2, 0, nc.sync)
                        if r == 3:
                            emit_xbar(2, 1, nc.sync)
                        if r == 4:
                            emit_mask_convert(3, ma3_i, mb3_i)
                            emit_smalls(3)
                            emit_nmA_pre(3)
                            emit_nmA(2)
                        if r == 5:
                            for j in range(NT):
                                emit_sta(3, j)
                        if r == 6:
                            emit_stb(3, 0)
                            emit_stb(3, 1)
                        if r == 7:
                            emit_comb_static(2)
                    if e == 2:
                        if r == 0:
                            emit_abd(3, nc.gpsimd)
                        if r == 2:
                            alloc_xops(3)
                            emit_xbar(3, 0, nc.sync)
                        if r == 3:
                            emit_xbar(3, 1, nc.sync)
                        if r == 5:
                            emit_nmA(3)
                        if r == 7:
                            emit_comb_static(3)
                emit_stage2(e - 1, NT - 3)
                emit_epi(e - 1, NT - 4)
                emit_stage2(e - 1, NT - 2)
                emit_epi(e - 1, NT - 3)
                emit_stage2(e - 1, NT - 1)
                emit_fix(e)

            slot(1)
            slot(2)
            slot(3)
            # drain: epi/out of e2, then stage2(3) (part1 before fix(3) lands)
            emit_epi(2, NT - 2)
            emit_epi(2, NT - 1)
            emit_out(2)
            emit_stage2(3, 0, part="1a")
            emit_stage2(3, 1, part="1a")
            emit_stage2(3, 2, part="1a")
            emit_stage2(3, 0, part="1b")
            emit_stage2(3, 1, part="1b")
            emit_stage2(3, 2, part="1b")
            emit_corr(3)
            emit_stage2(3, 0, part=2)
            for lt in range(1, NT):
                if lt >= 3:
                    emit_stage2(3, lt, part=1)
                emit_stage2(3, lt, part=2)
                emit_epi(3, lt - 1)
                if lt == 5:
                    emit_out(3, part=(0, 2))
                if lt == 7:
                    emit_out(3, part=(2, 4))
            emit_epi(3, NT - 1)
            emit_out(3, part=(3, 4))
    if not nc.is_finalized():
        nc.finalize()
    return nc


_NC = None


def _make_in_maps(a, b, mask_a, mask_b, temperature):
    temp = np.asarray(temperature, dtype=np.float32).reshape(1, 1)
    in_maps = []
    for i in range(NCORES):
        sl = slice(i * EPB, (i + 1) * EPB)
        in_maps.append(
            {
                "a": np.ascontiguousarray(np.asarray(a)[sl], dtype=np.float32),
                "b": np.ascontiguousarray(np.asarray(b)[sl], dtype=np.float32),
                "mask_a": np.ascontiguousarray(
                    np.asarray(mask_a)[sl], dtype=np.int32
                ),
                "mask_b": np.ascontiguousarray(
                    np.asarray(mask_b)[sl], dtype=np.int32
                ),
                "temperature": temp,
            }
        )
    return in_maps


def _gather(res):
    outs = [r["out"] for r in res.results]
    feature_a = np.concatenate([o[0] for o in outs], axis=0)
    feature_b = np.concatenate([o[1] for o in outs], axis=0)
    return (feature_a, feature_b)


def kernel(a, b, mask_a, mask_b, temperature):
    global _NC
    if _NC is None:
        _NC = build_nc()
    in_maps = _make_in_maps(a, b, mask_a, mask_b, temperature)
    res = run_bass_kernel_spmd(_NC, in_maps, core_ids=list(range(NCORES)))
    return _gather(res)


def kernel_traced(a, b, mask_a, mask_b, temperature, **kw):
    global _NC
    if _NC is None:
        _NC = build_nc()
    in_maps = _make_in_maps(a, b, mask_a, mask_b, temperature)
    res = run_bass_kernel_spmd(
        _NC, in_maps, core_ids=list(range(NCORES)), trace=True, **kw
    )
    return _gather(res), res


if __name__ == "__main__":
    import reference

    inputs = reference.setup_inputs()
    inputs = {k: np.asarray(v) for k, v in inputs.items()}
    exp_a, exp_b = reference.reference(**inputs)
    got_a, got_b = kernel(**inputs)
    for name, g, x in (("feature_a", got_a, exp_a), ("feature_b", got_b, exp_b)):
        x = np.asarray(x)
        rel = np.linalg.norm(g - x) / np.linalg.norm(x)
        print(f"{name}: rel={rel:.3e} max_abs={np.abs(g - x).max():.3e}")
